# revision 1
# baseline (speedup 1.0000x reference)
"""Bass/Trainium2 kernel for nn_CriterionSA (CAM/gridPAM CKA loss).

Self-contained: hardcodes shapes/sharding for the
B=16, C=256, H=W=80 problem on 8 NeuronCores.

Sharding:
  - PAM: 25 grid chunks -> cores 0..7 own 3 whole chunks each (chunks 3j..3j+2,
    all 16 samples local -> per-chunk [16,16] gram computed on-device).
    Chunk 24 is split 2 samples/core; its per-sample flattened PAM outputs are
    returned to the host, which computes that chunk's gram.
  - CAM: energy+softmax per-sample (2 samples/core, S and T), AllGather of the
    normalized transposed attention (gamma folded in), then each core computes
    the attention output restricted to its 800-position spatial slice for all
    16 samples and a partial [16,16] gram; host sums partials.
  - Grams on device use a column-interleaved SBUF layout (16 samples
    interleaved along free dim) + accumulating 128x128 matmuls whose 8
    diagonal 16x16 blocks are the gram contributions (host extracts/sums).
  - Final CKA centering/log on host from the tiny [16,16] grams.
"""

import os
import sys

import numpy as np

_REPO = "/opt/trn_rl_repo"
if _REPO not in sys.path:
    sys.path.insert(0, _REPO)

import concourse.bacc as bacc
import concourse.mybir as mybir
import concourse.tile as tile
from concourse import bass_utils

F32 = mybir.dt.float32
EXP = mybir.ActivationFunctionType.Exp
IDN = mybir.ActivationFunctionType.Identity
AX = mybir.AxisListType.X

NCORES = 8
B, C, H, W = 16, 256, 80, 80
CK = 32          # C // 8
NCH = 256        # spatial positions per grid chunk (16x16)
NSL = H * W // NCORES  # 800 spatial positions per core for CAM
TAU = 1.0

IN_SPECS = {
    "pamx":  (6, 2, 128, 4096),   # (t*3+ci, c-tile, c_low, b*256+m) natural chunk
    "pamxt": (6, 2, 128, 4096),   # (t*3+ci, m-tile, m_low, b*256+c) X^T + gamma*bv
    "c24x":  (2, 2, 2, 128, 256),  # (t, own-sample, c-tile, c_low, m)
    "c24xt": (2, 2, 2, 128, 256),  # (t, own-sample, m-tile, m_low, c)
    "camxt": (2, 2, 50, 128, 256),  # (t, own-sample, n-tile, n_low, c) raw X^T
    "camxn": (2, 16, 2, 128, 800),  # (t, b, c-tile, c_low, n-slice) natural
    "wqT":   (2, 128, 32),
    "wkT":   (2, 128, 32),
    "wvT":   (2, 128, 256),        # (gamma_pam * Wv)^T
    "bq4":   (128, 1),
    "i128":  (128, 128),
    "gicam": (128, 128),           # gamma_cam * I
}
OUT_SPECS = {
    "gpam": (6, 4, 128, 128),      # 4 partial accumulations of 16 supers each
    "gcam": (2, 26, 128, 128),     # 26 partial accumulations (13 per window)
    "c24r": (2, 2, 2, 128, 256),   # (t, own-sample, m-tile, m_low, c) PAM R^T
}


# --------------------------------------------------------------------------
# device program
# --------------------------------------------------------------------------

def _emit_softmax_attn_T(nc, sb, ep, eye_ap, n_i, tag):
    """From energy PSUM tile ep [128, 512] (two 256-wide row-blocks along free),
    produce (expE sbuf [128,512], dg sbuf [128,256]) where dg holds two 128x128
    diagonal blocks diag(eye_scale / Z). Softmax rows are the PARTITION dim of
    each 256-block; normalization uses exp(E - rowmax)."""
    nm = sb.tile([128, 2], F32, tag=f"nm{tag}", name=f"nm{tag}")
    nc.vector.tensor_reduce(
        nm[:], ep.rearrange("p (i j) -> p i j", i=2), AX,
        op=mybir.AluOpType.max, negate=True)
    expe = sb.tile([128, 512], F32, tag=f"expe{tag}", name=f"expe{tag}")
    zz = sb.tile([128, 2], F32, tag=f"zz{tag}", name=f"zz{tag}")
    for i in range(n_i):
        nc.scalar.activation(
            expe[:, i * 256:(i + 1) * 256], ep[:, i * 256:(i + 1) * 256],
            EXP, bias=nm[:, i:i + 1], scale=1.0, accum_out=zz[:, i:i + 1])
    rr = sb.tile([128, 2], F32, tag=f"rr{tag}", name=f"rr{tag}")
    nc.vector.reciprocal(rr[:, 0:n_i], zz[:, 0:n_i])
    dg = sb.tile([128, 256], F32, tag=f"dg{tag}", name=f"dg{tag}")
    for i in range(n_i):
        nc.vector.tensor_scalar_mul(
            dg[:, i * 128:(i + 1) * 128], eye_ap, rr[:, i:i + 1])
    return expe, dg


def _emit_pam_sample(nc, cst, sbs, psa, pso, q_sl, k_sl, xf, boff, xtp, xtoff,
                     ep2_pool, row_base=0):
    """One PAM attention sample. q_sl/k_sl: [32,256] APs (same base partition,
    = row_base). xf: 2 natural c-tiles; boff: free offset of this sample in xf.
    xtp: 2 m-tiles of X^T'; xtoff: free offset in xtp.
    Returns op_ PSUM tile [128, 512] = R^T, layout (m-tile 2)(c 256)."""
    ep2 = ep2_pool.tile([128, 512], F32, tag="ep2", name="ep2")
    for ib in range(2):
        nc.tensor.matmul(
            ep2[:, ib * 256:(ib + 1) * 256],
            lhsT=q_sl[:, ib * 128:(ib + 1) * 128], rhs=k_sl,
            start=True, stop=True, tile_position=(row_base, 0))
    expe, dg = _emit_softmax_attn_T(nc, sbs, ep2, cst["i128"][:], 2, "p")
    avp = psa.tile([128, 1024], F32, tag="avp", name="avp")
    # A^T (normalized) blocks: avp[:, jb*256+ib*128] = expE[ib-rows, jb-cols]^T * diag
    for jb in range(2):
        for ib in range(2):
            nc.tensor.matmul(
                avp[:, jb * 256 + ib * 128: jb * 256 + ib * 128 + 128],
                lhsT=expe[:, ib * 256 + jb * 128: ib * 256 + jb * 128 + 128],
                rhs=dg[:, ib * 128:(ib + 1) * 128], start=True, stop=True)
    # v^T = Xf^T @ (gamma Wv)^T
    for jb in range(2):
        for cb in range(2):
            nc.tensor.matmul(
                avp[:, 512 + jb * 256: 512 + (jb + 1) * 256],
                lhsT=xf[cb][:, boff + jb * 128: boff + jb * 128 + 128],
                rhs=cst["wvT"][cb][:], start=(cb == 0), stop=(cb == 1))
    av = sbs.tile([128, 1024], F32, tag="av", name="av")
    nc.scalar.copy(av[:, 0:512], avp[:, 0:512])
    nc.vector.tensor_copy(av[:, 512:1024], avp[:, 512:1024])
    op_ = pso.tile([128, 512], F32, tag="opam", name="opam")
    for mb in range(2):
        for jb in range(2):
            nc.tensor.matmul(
                op_[:, mb * 256:(mb + 1) * 256],
                lhsT=av[:, jb * 256 + mb * 128: jb * 256 + mb * 128 + 128],
                rhs=av[:, 512 + jb * 256: 512 + (jb + 1) * 256],
                start=(jb == 0), stop=False)
        # residual: += X^T + gamma*bv (prefolded on host)
        nc.tensor.matmul(
            op_[:, mb * 256:(mb + 1) * 256],
            lhsT=cst["i128"][:], rhs=xtp[mb][:, xtoff:xtoff + 256],
            start=False, stop=True)
    return op_


def _emit_program(nc, I, O):
    phases = os.environ.get("CRIT_PHASES", "012")
    with tile.TileContext(nc) as tc:
        cpool = tc.alloc_tile_pool(name="const", bufs=1)
        dram = tc.alloc_tile_pool(name="ccdram", bufs=1, space="DRAM")
        cst = {}
        for nm_ in ("wqT", "wkT", "wvT"):
            cst[nm_] = []
            for kb in range(2):
                t = cpool.tile(list(IN_SPECS[nm_][1:]), F32, name=f"{nm_}{kb}")
                nc.sync.dma_start(t[:], I[nm_][kb])
                cst[nm_].append(t)
        for nm_ in ("bq4", "i128", "gicam"):
            t = cpool.tile(list(IN_SPECS[nm_]), F32, name=nm_)
            nc.sync.dma_start(t[:], I[nm_][:])
            cst[nm_] = t

        atnb = dram.tile([8, 128, 256], F32, name="atnb")
        atng = dram.tile([8, 8, 128, 256], F32, name="atng", addr_space="Shared")

        for _rep in range(int(os.environ.get("CRIT_REPS", "1"))):
            _emit_body(tc, nc, I, O, cst, atnb, atng, phases)

        cpool.release()
        dram.release()


def _emit_body(tc, nc, I, O, cst, atnb, atng, phases):
        # ---------------- P0: CAM energy+attn for own samples ----------------
        with tc.tile_pool(name="p0sb", bufs=3) as sb0, \
                tc.tile_pool(name="p0ps", bufs=2, space="PSUM") as ps0:
            for t in range(2 if "0" in phases else 0):
                for bo in range(2):
                    eps = [ps0.tile([128, 256], F32, tag=f"ep{cb}", name=f"ep{cb}")
                           for cb in range(2)]
                    for nt in range(50):
                        xt = sb0.tile([128, 256], F32, tag="xt", name="xt")
                        nc.sync.dma_start(xt[:], I["camxt"][t, bo, nt])
                        for cb in range(2):
                            nc.tensor.matmul(
                                eps[cb][:],
                                lhsT=xt[:, cb * 128:(cb + 1) * 128], rhs=xt[:],
                                start=(nt == 0), stop=(nt == 49))
                    # softmax of (min - E) rows == softmax(max_d E - E) rows
                    mn = sb0.tile([128, 2], F32, tag="mnc", name="mnc")
                    for cb in range(2):
                        nc.vector.tensor_reduce(
                            mn[:, cb:cb + 1], eps[cb][:], AX,
                            op=mybir.AluOpType.min)
                    expe = sb0.tile([128, 512], F32, tag="expec", name="expec")
                    zz = sb0.tile([128, 2], F32, tag="zzc", name="zzc")
                    for cb in range(2):
                        nc.scalar.activation(
                            expe[:, cb * 256:(cb + 1) * 256],
                            eps[cb][:],
                            EXP, bias=mn[:, cb:cb + 1], scale=-1.0,
                            accum_out=zz[:, cb:cb + 1])
                    rr = sb0.tile([128, 2], F32, tag="rrc", name="rrc")
                    nc.vector.reciprocal(rr[:], zz[:])
                    dgc = sb0.tile([128, 256], F32, tag="dgc", name="dgc")
                    for cb in range(2):
                        nc.vector.tensor_scalar_mul(
                            dgc[:, cb * 128:(cb + 1) * 128],
                            cst["gicam"][:], rr[:, cb:cb + 1])
                    atc = ps0.tile([128, 512], F32, tag="atc", name="atc")
                    for dt in range(2):
                        for cb in range(2):
                            nc.tensor.matmul(
                                atc[:, dt * 256 + cb * 128: dt * 256 + cb * 128 + 128],
                                lhsT=expe[:, cb * 256 + dt * 128: cb * 256 + dt * 128 + 128],
                                rhs=dgc[:, cb * 128:(cb + 1) * 128],
                                start=True, stop=True)
                    atcs = sb0.tile([128, 512], F32, tag="atcs", name="atcs")
                    nc.vector.tensor_copy(atcs[:], atc[:])
                    for dt in range(2):
                        nc.sync.dma_start(
                            atnb[bo * 4 + t * 2 + dt], atcs[:, dt * 256:(dt + 1) * 256])

        if "0" in phases:
            nc.gpsimd.collective_compute(
                "AllGather", mybir.AluOpType.bypass,
                replica_groups=[list(range(NCORES))],
                ins=[atnb.opt()], outs=[atng.opt()])

        # ---------------- P1: PAM chunks ----------------
        with tc.tile_pool(name="pxf", bufs=2) as sbx, \
                tc.tile_pool(name="pxt", bufs=1) as sbxt, \
                tc.tile_pool(name="pX", bufs=1) as sbX, \
                tc.tile_pool(name="pqk", bufs=1) as sbqk, \
                tc.tile_pool(name="psmall", bufs=2) as sbs, \
                tc.tile_pool(name="qkps", bufs=1, space="PSUM") as psq, \
                tc.tile_pool(name="eps", bufs=1, space="PSUM") as pse, \
                tc.tile_pool(name="avps", bufs=1, space="PSUM") as psa, \
                tc.tile_pool(name="ops", bufs=2, space="PSUM") as pso, \
                tc.tile_pool(name="gps", bufs=1, space="PSUM") as psg:
            for u in range(6 if "1" in phases else 0):
                xf = []
                for cb in range(2):
                    xft = sbx.tile([128, 4096], F32, tag=f"xf{cb}", name=f"xf{cb}")
                    nc.sync.dma_start(xft[:], I["pamx"][u, cb])
                    xf.append(xft)
                xtp = []
                for mt in range(2):
                    xtt = sbxt.tile([128, 4096], F32, tag=f"xtp{mt}", name=f"xtp{mt}")
                    nc.sync.dma_start(xtt[:], I["pamxt"][u, mt])
                    xtp.append(xtt)
                # q/k passes (samples col-packed 4-wide, 512-wide windows)
                qtb = sbqk.tile([128, 1024], F32, tag="qtb", name="qtb")
                ktb = sbqk.tile([128, 1024], F32, tag="ktb", name="ktb")
                for which, wt, dst in (("q", "wqT", qtb), ("k", "wkT", ktb)):
                    qp = psq.tile([128, 1024], F32, tag="qkp", name="qkp")
                    for w in range(8):
                        r_ = 32 * (w % 4)
                        fo = (w // 4) * 512
                        for kb in range(2):
                            nc.tensor.matmul(
                                qp[r_:r_ + 32, fo:fo + 512],
                                lhsT=cst[wt][kb][:],
                                rhs=xf[kb][:, w * 512:(w + 1) * 512],
                                start=(kb == 0), stop=(kb == 1),
                                tile_position=(0, r_))
                    if which == "q":
                        nc.scalar.activation(dst[:], qp[:], IDN,
                                             bias=cst["bq4"][:], scale=1.0)
                    else:
                        nc.scalar.copy(dst[:], qp[:])
                X = sbX.tile([128, 8192], F32, tag="X", name="X")
                for b in range(16):
                    w = b // 2
                    rb = 32 * (w % 4)
                    fo = (w // 4) * 512 + (b % 2) * 256
                    op_ = _emit_pam_sample(
                        nc, cst, sbs, psa, pso,
                        qtb[rb:rb + 32, fo:fo + 256], ktb[rb:rb + 32, fo:fo + 256],
                        xf, b * 256, xtp, b * 256, pse, row_base=rb)
                    nc.vector.tensor_copy(
                        X.rearrange("p (mt d b2) -> p mt d b2", mt=2, b2=16)[:, :, :, b],
                        op_.rearrange("p (mt d) -> p mt d", mt=2))
                for g in range(4):
                    gp = psg.tile([128, 128], F32, tag="gp", name="gp")
                    for si in range(16):
                        s = g * 16 + si
                        nc.tensor.matmul(
                            gp[:], lhsT=X[:, s * 128:(s + 1) * 128],
                            rhs=X[:, s * 128:(s + 1) * 128],
                            start=(si == 0), stop=(si == 15))
                    gps = sbs.tile([128, 128], F32, tag="gpsb", name="gpsb")
                    nc.scalar.copy(gps[:], gp[:])
                    nc.sync.dma_start(O["gpam"][u, g], gps[:])

            # chunk 24: 2 own samples, R^T straight to DRAM
            for t in range(2 if "1" in phases else 0):
                for bo in range(2):
                    xf4 = []
                    for cb in range(2):
                        x4 = sbs.tile([128, 256], F32, tag=f"xf4{cb}", name=f"xf4{cb}")
                        nc.sync.dma_start(x4[:], I["c24x"][t, bo, cb])
                        xf4.append(x4)
                    xtp4 = []
                    for mt in range(2):
                        x4t = sbs.tile([128, 256], F32, tag=f"xt4{mt}", name=f"xt4{mt}")
                        nc.sync.dma_start(x4t[:], I["c24xt"][t, bo, mt])
                        xtp4.append(x4t)
                    qtb4 = sbs.tile([32, 256], F32, tag="qtb4", name="qtb4")
                    ktb4 = sbs.tile([32, 256], F32, tag="ktb4", name="ktb4")
                    for which, wt, dst in (("q", "wqT", qtb4), ("k", "wkT", ktb4)):
                        qp4 = psq.tile([128, 1024], F32, tag="qkp", name="qkp")
                        for kb in range(2):
                            nc.tensor.matmul(
                                qp4[0:32, 0:256], lhsT=cst[wt][kb][:],
                                rhs=xf4[kb][:], start=(kb == 0), stop=(kb == 1))
                        if which == "q":
                            nc.scalar.activation(dst[:], qp4[0:32, 0:256], IDN,
                                                 bias=cst["bq4"][0:32, :], scale=1.0)
                        else:
                            nc.scalar.copy(dst[:], qp4[0:32, 0:256])
                    op4 = _emit_pam_sample(
                        nc, cst, sbs, psa, pso, qtb4[:], ktb4[:],
                        xf4, 0, xtp4, 0, pse)
                    op4s = sbs.tile([128, 512], F32, tag="op4s", name="op4s")
                    nc.vector.tensor_copy(op4s[:], op4[:])
                    for mt in range(2):
                        nc.sync.dma_start(
                            O["c24r"][t, bo, mt], op4s[:, mt * 256:(mt + 1) * 256])

        # ---------------- P2: CAM out-slice + partial grams ----------------
        with tc.tile_pool(name="c2at", bufs=1) as sb2a, \
                tc.tile_pool(name="c2x", bufs=1) as sb2x, \
                tc.tile_pool(name="c2n", bufs=3) as sb2n, \
                tc.tile_pool(name="c2ops", bufs=2, space="PSUM") as ps2o, \
                tc.tile_pool(name="c2gps", bufs=1, space="PSUM") as ps2g:
            for t in range(2 if "2" in phases else 0):
                atn = sb2a.tile([128, 8192], F32, tag="atn", name="atn")
                for b in range(16):
                    for dt in range(2):
                        nc.sync.dma_start(
                            atn[:, (b * 2 + dt) * 256:(b * 2 + dt + 1) * 256],
                            atng[b // 2, (b % 2) * 4 + t * 2 + dt])
                for w in range(2):
                    Xw = sb2x.tile([128, 12800], F32, tag="Xw", name="Xw")
                    for b in range(16):
                        xn = []
                        for cb in range(2):
                            xnt = sb2n.tile([128, 400], F32, tag=f"xn{cb}", name=f"xn{cb}")
                            nc.sync.dma_start(
                                xnt[:], I["camxn"][t, b, cb, :, w * 400:(w + 1) * 400])
                            xn.append(xnt)
                        ocp = ps2o.tile([128, 1024], F32, tag="ocp", name="ocp")
                        for cb in range(2):
                            for dt in range(2):
                                nc.tensor.matmul(
                                    ocp[:, cb * 512:cb * 512 + 400],
                                    lhsT=atn[:, (b * 2 + dt) * 256 + cb * 128:
                                             (b * 2 + dt) * 256 + cb * 128 + 128],
                                    rhs=xn[dt][:], start=(dt == 0), stop=False)
                            nc.tensor.matmul(
                                ocp[:, cb * 512:cb * 512 + 400],
                                lhsT=cst["i128"][:], rhs=xn[cb][:],
                                start=False, stop=True)
                        dst = Xw.rearrange(
                            "p (cb n b2) -> p cb n b2", cb=2, b2=16)[:, :, :, b]
                        src = ocp.rearrange("p (cb n) -> p cb n", cb=2)[:, :, 0:400]
                        if b % 2 == 0:
                            nc.vector.tensor_copy(dst, src)
                        else:
                            nc.scalar.copy(dst, src)
                    # 100 supers per window -> 12 partials of 8 + 1 of 4
                    sidx = 0
                    for g in range(13):
                        n_s = 8 if g < 12 else 4
                        gcp = ps2g.tile([128, 128], F32, tag="gcp", name="gcp")
                        for si in range(n_s):
                            s = sidx + si
                            nc.tensor.matmul(
                                gcp[:], lhsT=Xw[:, s * 128:(s + 1) * 128],
                                rhs=Xw[:, s * 128:(s + 1) * 128],
                                start=(si == 0), stop=(si == n_s - 1))
                        sidx += n_s
                        gcs = sb2n.tile([128, 128], F32, tag="gcs", name="gcs")
                        nc.scalar.copy(gcs[:], gcp[:])
                        nc.sync.dma_start(O["gcam"][t, w * 13 + g], gcs[:])


_PROG = None


def _get_prog():
    global _PROG
    if _PROG is None:
        nc = bacc.Bacc("TRN2", target_bir_lowering=False, debug=False,
                       num_devices=NCORES)
        I = {n: nc.dram_tensor(n, list(s), F32, kind="ExternalInput").ap()
             for n, s in IN_SPECS.items()}
        O = {n: nc.dram_tensor(n, list(s), F32, kind="ExternalOutput").ap()
             for n, s in OUT_SPECS.items()}
        _emit_program(nc, I, O)
        nc.compile()
        _PROG = nc
    return _PROG


# --------------------------------------------------------------------------
# host side
# --------------------------------------------------------------------------

def _make_in_maps(feat_S, feat_T, Wq, bq, Wk, bk, Wv, bv, gammacam, gammapam):
    gp = float(np.asarray(gammapam).reshape(-1)[0])
    gc = float(np.asarray(gammacam).reshape(-1)[0])
    feats = [np.ascontiguousarray(feat_S, np.float32),
             np.ascontiguousarray(feat_T, np.float32)]
    featsT = [np.ascontiguousarray(f.transpose(0, 2, 3, 1)) for f in feats]
    gbv = (gp * np.asarray(bv, np.float32)).astype(np.float32)
    featsTp = [(fT + gbv[None, None, None, :]).astype(np.float32) for fT in featsT]

    consts = {
        "wqT": np.ascontiguousarray(np.asarray(Wq, np.float32).T.reshape(2, 128, CK)),
        "wkT": np.ascontiguousarray(np.asarray(Wk, np.float32).T.reshape(2, 128, CK)),
        "wvT": np.ascontiguousarray(
            (gp * np.asarray(Wv, np.float32)).T.reshape(2, 128, C)),
        "bq4": np.ascontiguousarray(np.tile(np.asarray(bq, np.float32), 4)[:, None]),
        "i128": np.eye(128, dtype=np.float32),
        "gicam": (gc * np.eye(128)).astype(np.float32),
    }

    in_maps = []
    for j in range(NCORES):
        m = dict(consts)
        pamx = np.empty((6, 2, 128, 4096), np.float32)
        pamxt = np.empty((6, 2, 128, 4096), np.float32)
        for t in range(2):
            for ci in range(3):
                c = 3 * j + ci
                gi, gj = divmod(c, 5)
                blk = feats[t][:, :, gi * 16:gi * 16 + 16, gj * 16:gj * 16 + 16]
                pamx[t * 3 + ci] = (blk.reshape(B, 2, 128, 256)
                                    .transpose(1, 2, 0, 3).reshape(2, 128, 4096))
                blkT = featsTp[t][:, gi * 16:gi * 16 + 16, gj * 16:gj * 16 + 16, :]
                pamxt[t * 3 + ci] = (blkT.reshape(B, 2, 128, 256)
                                     .transpose(1, 2, 0, 3).reshape(2, 128, 4096))
        m["pamx"] = pamx
        m["pamxt"] = pamxt

        c24x = np.empty((2, 2, 2, 128, 256), np.float32)
        c24xt = np.empty((2, 2, 2, 128, 256), np.float32)
        camxt = np.empty((2, 2, 50, 128, 256), np.float32)
        for t in range(2):
            for bo in range(2):
                b = 2 * j + bo
                c24x[t, bo] = feats[t][b, :, 64:80, 64:80].reshape(2, 128, 256)
                c24xt[t, bo] = featsTp[t][b, 64:80, 64:80, :].reshape(2, 128, 256)
                camxt[t, bo] = featsT[t][b].reshape(50, 128, 256)
        m["c24x"] = c24x
        m["c24xt"] = c24xt
        m["camxt"] = camxt

        camxn = np.empty((2, 16, 2, 128, 800), np.float32)
        for t in range(2):
            for b in range(B):
                camxn[t, b] = feats[t][b, :, 10 * j:10 * j + 10, :].reshape(2, 128, 800)
        m["camxn"] = camxn
        in_maps.append(m)
    return in_maps


def _diag16(gfull):
    """gfull: [..., 128, 128] partials; f64-sum partials then diagonal blocks."""
    gf = gfull.astype(np.float64).reshape(-1, 128, 128).sum(axis=0)
    g = np.zeros((16, 16), np.float64)
    for r in range(8):
        g += gf[16 * r:16 * r + 16, 16 * r:16 * r + 16]
    return g


def _cka_loss(KS, KT):
    def cgram(K):
        rm = K.mean(axis=1, keepdims=True)
        cm = K.mean(axis=0, keepdims=True)
        return K - rm - cm + K.mean()
    cX, cY = cgram(KS), cgram(KT)
    hsic = float((cX * cY).sum())
    v1 = float(np.sqrt((cX * cX).sum()))
    v2 = float(np.sqrt((cY * cY).sum()))
    return -np.log(np.abs(hsic / (v1 * v2)) + 1e-8)


def _postprocess(results):
    losses = []
    for c in range(24):
        j, ci = divmod(c, 3)
        res = results[j]
        KS = _diag16(res["gpam"][ci])
        KT = _diag16(res["gpam"][3 + ci])
        losses.append(_cka_loss(KS, KT))
    # chunk 24 on host
    FS = np.empty((B, 2 * 128 * 256), np.float32)
    FT = np.empty((B, 2 * 128 * 256), np.float32)
    for j in range(NCORES):
        for bo in range(2):
            FS[2 * j + bo] = results[j]["c24r"][0, bo].reshape(-1)
            FT[2 * j + bo] = results[j]["c24r"][1, bo].reshape(-1)
    FS = FS.astype(np.float64)
    FT = FT.astype(np.float64)
    KS24 = FS @ FS.T
    KT24 = FT @ FT.T
    losses.append(_cka_loss(KS24, KT24))
    loss_PAM = float(np.mean(losses))

    KSc = np.zeros((16, 16), np.float64)
    KTc = np.zeros((16, 16), np.float64)
    for j in range(NCORES):
        KSc += _diag16(results[j]["gcam"][0])
        KTc += _diag16(results[j]["gcam"][1])
    loss_CAM = float(_cka_loss(KSc, KTc))
    return np.float32(loss_CAM), np.float32(loss_PAM)


def _run_sim(nc, in_maps):
    from concourse.bass_interp import MultiCoreSim
    sim = MultiCoreSim(nc, num_cores=NCORES)
    cores = list(sim.cores.values())
    for j, core in enumerate(cores):
        for name, arr in in_maps[j].items():
            core.tensor(name)[:] = arr
    sim.simulate()
    return [{n: core.tensor(n).copy() for n in OUT_SPECS} for core in cores]


_LAST_EXEC_NS = None


def kernel(**inputs):
    global _LAST_EXEC_NS
    nc = _get_prog()
    in_maps = _make_in_maps(**{k: np.asarray(v) for k, v in inputs.items()})
    if os.environ.get("CRIT_BACKEND", "hw") == "sim":
        results = _run_sim(nc, in_maps)
    else:
        res = bass_utils.run_bass_kernel_spmd(
            nc, in_maps, core_ids=list(range(NCORES)),
            trace=os.environ.get("CRIT_TRACE", "0") == "1")
        results = res.results
        _LAST_EXEC_NS = res.exec_time_ns
    return _postprocess(results)

